# revision 1
# baseline (speedup 1.0000x reference)
"""Causal multi-head attention block (qkv proj + attention + out proj) on 8
Trainium2 NeuronCores.

Sharding: core c = 2*b + hg handles batch b (of 4) and head-group hg (8 of 16
heads).  Each core computes qkv for its heads, causal attention, and a partial
out-projection (its 512 rows of w_out); the host sums the two head-group
partials per batch.

Per-core layout (all matmuls fp32r):
  - x [T, DIM] is PE-transposed into xT [DIM, T] per t-quarter; Q^T/K^T come
    out of the projection as head-pair tiles [128 = 2 heads x 64, t]; V in
    natural [t, c] layout augmented with a ones column per head (V_aug), so
    P @ V_aug accumulates the numerator and the softmax denominator together
    (softmax runs without max-subtraction: scores ~ N(0,1), exp safe in fp32).
  - scores are computed transposed, S^T[k, q], two heads concurrently via PE
    row tiling (K=64 each) into one two-bank PSUM tile; exp (scale fused) is
    one ACT op per pair, narrowed on diagonal blocks; causal masking is a 0/1
    multiply on one 128-wide strip + a zero-fill left of it.
  - normalization per head-pair: denominator rows round-trip through DRAM to
    reshape [2,512] -> [128,8] so the DVE reciprocal runs 8 elems/lane, then a
    partition-broadcast DMA load and one in-place DVE multiply.
  - emission is phase-interleaved: qkv quarter q+1 and out_proj q-1 (dense PE
    work) are pumped into attention block q (ACT/exp-bound) at iteration
    granularity to keep the PE dense and HAM-warm; the qkv weight/staging
    pools close after phase 2 to give the tail phase larger late pools.
"""

import sys

if "/opt/trn_rl_repo" not in sys.path:
    sys.path.insert(0, "/opt/trn_rl_repo")

import numpy as np

import concourse.bass as bass
import concourse.mybir as mybir
import concourse.tile as tile
from concourse import bacc
from concourse.masks import make_identity
from concourse.bass_utils import run_bass_kernel_spmd

DIM = 1024
N_HEAD = 16
HD = 64
B, T = 4, 2048
HG = 8          # heads per core
CQ = HG * HD    # 512 feature columns per group
NCORES = 8
NT = T // 128   # 16 t-subtiles
NQ = T // 512   # 4 quarters / q-blocks

f32 = mybir.dt.float32
f32r = mybir.dt.float32r
Exp = mybir.ActivationFunctionType.Exp


def build_nc(apply_mask=True):
    nc = bacc.Bacc(None, target_bir_lowering=False)
    x_d = nc.declare_dram_parameter("x", [T, DIM], f32, isOutput=False)
    wqk_d = nc.declare_dram_parameter("wqk", [DIM, 2 * CQ], f32, isOutput=False)
    wv_d = nc.declare_dram_parameter("wv", [DIM, CQ], f32, isOutput=False)
    wo_d = nc.declare_dram_parameter("wo", [CQ, DIM], f32, isOutput=False)
    mv_d = nc.declare_dram_parameter("maskv", [128, NT], f32, isOutput=False)
    out_d = nc.declare_dram_parameter("out", [T, DIM], f32, isOutput=True)

    from contextlib import ExitStack

    with tile.TileContext(nc) as tc:
        qkv_scope = ExitStack()
        with tc.tile_pool(name="pp", bufs=1) as pp, \
             tc.tile_pool(name="qtp", bufs=2) as qtp, \
             tc.tile_pool(name="p_p", bufs=3) as p_p, \
             tc.tile_pool(name="at_p", bufs=1) as at_p, \
             tc.tile_pool(name="den_p", bufs=2) as den_p, \
             tc.tile_pool(name="rec_p", bufs=1) as rec_p, \
             tc.tile_pool(name="bcs_p", bufs=1) as bcs_p, \
             tc.tile_pool(name="out_p", bufs=2) as out_p, \
             tc.tile_pool(name="dram_p", bufs=2, space="DRAM") as dram_p, \
             tc.tile_pool(name="ps_aux", bufs=2, space="PSUM") as ps_aux, \
             tc.tile_pool(name="ps_s", bufs=2, space="PSUM") as ps_s, \
             tc.tile_pool(name="ps_pv", bufs=1, space="PSUM") as ps_pv:
            xrow_p = qkv_scope.enter_context(tc.tile_pool(name="xrow", bufs=2))
            xT_p = qkv_scope.enter_context(tc.tile_pool(name="xT", bufs=1))
            wwp = qkv_scope.enter_context(tc.tile_pool(name="wwp", bufs=1))

            # ---- constants ----
            ident32 = pp.tile([128, 128], f32, name="ident32", tag="ident32")
            make_identity(nc, ident32)
            ident = pp.tile([128, 128], f32r, name="ident", tag="ident")
            nc.vector.tensor_copy(ident, ident32)
            # one 128x128 causal strip: keep where q_local >= k_local
            dstrip = pp.tile([128, 128], f32, name="dstrip", tag="dstrip")
            nc.gpsimd.memset(dstrip, 1.0)
            nc.gpsimd.affine_select(
                out=dstrip, in_=dstrip, compare_op=mybir.AluOpType.is_ge,
                fill=0.0, base=0, pattern=[[1, 128]], channel_multiplier=-1)
            zerosr = pp.tile([128, 384], f32r, name="zerosr", tag="zerosr")
            nc.vector.memset(zerosr.bitcast(f32), 0.0)
            onescol = pp.tile([128, HG], f32, name="onescol", tag="onescol")
            nc.vector.memset(onescol, 1.0)
            mv_sb = pp.tile([128, NT], f32, name="maskv_sb", tag="maskv_sb")
            nc.sync.dma_start(out=mv_sb, in_=mv_d[:, :])

            # ---- persistent tensors ----
            kt = [pp.tile([128, T], f32r, name=f"kt{m}", tag=f"kt{m}") for m in range(4)]
            vaug = [pp.tile([128, HG * 65], f32r, name=f"vaug{t}", tag=f"vaug{t}")
                    for t in range(NT)]
            wo_sb = [pp.tile([128, DIM], f32r, name=f"wo{m}", tag=f"wo{m}")
                     for m in range(4)]
            wqk_sb = [wwp.tile([128, 2 * CQ], f32r, name=f"wqk{k}", tag=f"wqk{k}")
                      for k in range(8)]
            wv_sb = [wwp.tile([128, CQ], f32r, name=f"wv{k}", tag=f"wv{k}")
                     for k in range(8)]
            for m in range(4):
                nc.sync.dma_start(out=wo_sb[m],
                                  in_=wo_d[m * 128:(m + 1) * 128, :].bitcast(f32r))
            for k in range(8):
                nc.sync.dma_start(out=wqk_sb[k],
                                  in_=wqk_d[k * 128:(k + 1) * 128, :].bitcast(f32r))
                nc.sync.dma_start(out=wv_sb[k],
                                  in_=wv_d[k * 128:(k + 1) * 128, :].bitcast(f32r))

            qt_cur = {}    # quarter -> [4 pair tiles [128, 512]]
            ats_cur = {}   # qb -> [4 pair tiles [128, 512]]

            # ---------- qkv quarter units (each closure ~1-2 us of PE) ----------
            def qkv_units(q):
                units = []
                xts = [xT_p.tile([128, 512], f32r, name=f"xt{kb}", tag=f"xt{kb}")
                       for kb in range(8)]

                def xt_unit(ti):
                    # load 128 rows of x, PE-transpose into the 8 xT tiles
                    xr = xrow_p.tile([128, DIM], f32r, name="xr", tag="xr")
                    t0 = (q * 4 + ti) * 128
                    nc.sync.dma_start(out=xr, in_=x_d[t0:t0 + 128, :].bitcast(f32r))
                    for kb in range(8):
                        pst = ps_aux.tile([128, 128], f32r, name="pst", tag="aux")
                        nc.tensor.transpose(
                            pst, xr[:, kb * 128:(kb + 1) * 128], ident)
                        nc.vector.tensor_copy(
                            xts[kb][:, ti * 128:(ti + 1) * 128], pst)
                for ti in range(4):
                    units.append(lambda ti=ti: xt_unit(ti))

                qt_cur[q] = [None] * 4

                def qk_unit(m):
                    pq = ps_aux.tile([128, 512], f32, name="mm", tag="aux")
                    for kb in range(8):
                        nc.tensor.matmul(
                            pq, wqk_sb[kb][:, m * 128:(m + 1) * 128], xts[kb],
                            start=(kb == 0), stop=(kb == 7))
                    if m < 4:
                        qtile = qtp.tile([128, 512], f32r, name=f"qt{m}", tag=f"qt{m}")
                        nc.vector.tensor_copy(qtile, pq)
                        qt_cur[q][m] = qtile
                    else:
                        nc.vector.tensor_copy(
                            kt[m - 4][:, q * 512:(q + 1) * 512], pq)
                for m in range(8):
                    units.append(lambda m=m: qk_unit(m))

                def v_unit(ti):
                    pv = ps_aux.tile([128, 512], f32, name="mm", tag="aux")
                    for kb in range(8):
                        nc.tensor.matmul(
                            pv, xts[kb][:, ti * 128:(ti + 1) * 128], wv_sb[kb],
                            start=(kb == 0), stop=(kb == 7))
                    vt = vaug[q * 4 + ti]
                    vt3 = vt.rearrange("p (h w) -> p h w", w=65)
                    nc.vector.tensor_copy(
                        vt3[:, :, 0:64], pv.rearrange("p (h w) -> p h w", w=64))
                    nc.vector.tensor_copy(
                        vt3[:, :, 64:65], onescol.rearrange("p (h w) -> p h w", w=1))
                    nc.vector.tensor_scalar_mul(
                        vt, vt, mv_sb[:, (q * 4 + ti):(q * 4 + ti + 1)])
                for ti in range(4):
                    units.append(lambda ti=ti: v_unit(ti))
                return units

            # ---------- out_proj units for one q-block ----------
            def outproj_units(qb):
                units = []

                def op_unit(ti, nb):
                    ats = ats_cur[qb]
                    po = ps_aux.tile([128, 512], f32, name="mm", tag="aux")
                    for m in range(4):
                        nc.tensor.matmul(
                            po, ats[m][:, ti * 128:(ti + 1) * 128],
                            wo_sb[m][:, nb * 512:(nb + 1) * 512],
                            start=(m == 0), stop=(m == 3))
                    ob = out_p.tile([128, 512], f32, name="ob", tag="ob")
                    nc.vector.tensor_copy(ob, po)
                    t0 = (qb * 4 + ti) * 128
                    nc.sync.dma_start(
                        out=out_d[t0:t0 + 128, nb * 512:(nb + 1) * 512], in_=ob)
                for ti in range(4):
                    for nb in range(2):
                        units.append(lambda ti=ti, nb=nb: op_unit(ti, nb))
                return units

            # ---------- attention pair tasks + phase driver ----------
            AluAdd = mybir.AluOpType.add
            spill_dram = {}
            d1_cur = {}

            late = None

            def att_pair(qb, m, part, pump):
                """Emit one pair's k-loop. part: None=full, "A"=k<12 (spill),
                "B"=k>=12 (merge with spilled partial)."""
                nk = 4 * (qb + 1)
                k0, k1 = {"A": (0, 12), "B": (12, nk), None: (0, nk)}[part]
                pvp = ps_pv.tile([65, 1024], f32, name="pv", tag="pv")

                def pv_mms(pk, pt, w0, stop):
                    # masked q-columns [0:w0) of this k-tile are exactly zero:
                    # skip them; PSUM accumulation keeps their prior value
                    nc.tensor.matmul(
                        pvp[:, w0:512],
                        vaug[pk][:, (2 * m) * 65:(2 * m + 1) * 65],
                        pt[:, w0:512], start=(pk == k0), stop=stop)
                    nc.tensor.matmul(
                        pvp[:, 512 + w0:1024],
                        vaug[pk][:, (2 * m + 1) * 65:(2 * m + 2) * 65],
                        pt[:, 512 + w0:1024], start=(pk == k0), stop=stop)

                prev = None
                for kti in range(k0, k1):
                    j = kti - 4 * qb
                    w0 = 128 * j if j > 0 else 0
                    sp = ps_s.tile([128, 1024], f32, name="s", tag="s")
                    nc.tensor.matmul(
                        sp[:, w0:512],
                        kt[m][0:64, kti * 128:(kti + 1) * 128],
                        qt_cur[qb][m][0:64, w0:512], start=True, stop=True)
                    nc.tensor.matmul(
                        sp[:, 512 + w0:1024],
                        kt[m][64:128, kti * 128:(kti + 1) * 128],
                        qt_cur[qb][m][64:128, w0:512], start=True, stop=True)
                    pump()
                    ppt = (late["p"].tile([128, 1024], f32r, name="p", tag="lp")
                           if late else
                           p_p.tile([128, 1024], f32r, name="p", tag="p"))
                    p3 = ppt.rearrange("p (h w) -> p h w", w=512)
                    s3 = sp.rearrange("p (h w) -> p h w", w=512)
                    if j < 0:
                        nc.scalar.activation(p3, s3, Exp, scale=0.125)
                    else:
                        nc.scalar.activation(
                            p3[:, :, w0:512], s3[:, :, w0:512], Exp, scale=0.125)
                        for h in range(2):
                            nc.vector.tensor_mul(
                                ppt[:, h * 512 + w0:h * 512 + w0 + 128],
                                ppt[:, h * 512 + w0:h * 512 + w0 + 128],
                                dstrip)
                    if prev is not None:
                        pv_mms(*prev, stop=False)
                    prev = (kti, ppt, w0)
                    pump()
                pv_mms(*prev, stop=True)

                if part == "A":
                    st = spill_p.tile([65, 1024], f32, name="spst", tag="spst")
                    nc.vector.tensor_copy(st, pvp)
                    dsp = dram_p.tile([65, 1024], f32, name=f"dsp{m}", tag=f"dsp{m}")
                    nc.sync.dma_start(out=dsp, in_=st)
                    spill_dram[m] = dsp
                    return
                atm = ((late["at"] if late else at_p)
                       .tile([128, 512], f32r, name=f"at{m}", tag=f"at{m}"))
                ats_cur[qb][m] = atm
                d1 = dram_p.tile([2, 512], f32, name="d1", tag="d1")
                if part == "B":
                    st = spill_p.tile([65, 1024], f32, name="spst", tag="spst")
                    nc.sync.dma_start(out=st, in_=spill_dram[m])
                    nc.vector.tensor_tensor(
                        atm[0:64, :], pvp[0:64, 0:512], st[0:64, 0:512], AluAdd)
                    nc.vector.tensor_tensor(
                        atm[64:128, :], pvp[0:64, 512:1024],
                        st[0:64, 512:1024], AluAdd)
                    for h in range(2):
                        dn = den_p.tile([1, 512], f32, name="dn", tag="dn")
                        nc.vector.tensor_tensor(
                            dn, pvp[64:65, h * 512:(h + 1) * 512],
                            st[64:65, h * 512:(h + 1) * 512], AluAdd)
                        nc.sync.dma_start(
                            out=d1[h:h + 1, :], in_=dn)
                else:
                    for h in range(2):
                        dn = den_p.tile([1, 512], f32, name="dn", tag="dn")
                        nc.vector.tensor_copy(
                            dn, pvp[64:65, h * 512:(h + 1) * 512])
                        nc.sync.dma_start(
                            out=d1[h:h + 1, :], in_=dn)
                    nc.vector.tensor_copy(atm[0:64, :], pvp[0:64, 0:512])
                    nc.vector.tensor_copy(atm[64:128, :], pvp[0:64, 512:1024])
                # per-pair normalization: reciprocal at 8 elems/lane via DRAM
                # reshape, then partition-broadcast multiply
                den128 = rec_p.tile([128, 8], f32, name="den128", tag="den128")
                nc.sync.dma_start(
                    out=den128,
                    in_=d1.rearrange("i w -> (i w)").rearrange("(p c) -> p c", c=8))
                rec128 = rec_p.tile([128, 8], f32, name="rec128", tag="rec128")
                nc.vector.reciprocal(rec128, den128)
                d2 = dram_p.tile([2, 512], f32, name="d2", tag="d2")
                nc.sync.dma_start(
                    out=d2.rearrange("i w -> (i w)").rearrange("(p c) -> p c", c=8),
                    in_=rec128)
                bcs = ((late["bcs"] if late else bcs_p)
                       .tile([128, 512], f32, name="bcs", tag="bcs"))
                for h in range(2):
                    nc.sync.dma_start(
                        out=bcs[h * 64:(h + 1) * 64, :],
                        in_=d2[h:h + 1, :].partition_broadcast(64))
                nc.vector.tensor_mul(atm, atm, bcs)

            def run_phase(tasks, fillers, n_units):
                """tasks: closures taking pump(); fillers pumped proportionally."""
                nf = len(fillers)
                state = {"fi": 0, "ai": 0}

                def pump():
                    state["ai"] += 1
                    while state["fi"] * n_units < state["ai"] * nf \
                            and state["fi"] < nf:
                        fillers[state["fi"]]()
                        state["fi"] += 1
                for t in tasks:
                    t(pump)
                while state["fi"] < nf:
                    fillers[state["fi"]]()
                    state["fi"] += 1

            # ---------------- emission schedule ----------------
            for u in qkv_units(0):
                u()
            for qb in range(NQ):
                ats_cur[qb] = [None] * 4

            def phase_tasks(qb, part):
                def mk(m):
                    def t(pump):
                        att_pair(qb, m, part, pump)
                    return t
                return [mk(m) for m in range(4)]


            # phase 0: att(0) + qkv(1)
            run_phase(phase_tasks(0, None),
                      qkv_units(1), 32)
            # phase 1: att(1) + op(0) + qkv(2)
            run_phase(phase_tasks(1, None),
                      outproj_units(0) + qkv_units(2), 64)
            # phase 2: att(2) + op(1) + qkv(3), then att(3) pair 0 after
            # the qkv(3) fillers have all been emitted
            run_phase(phase_tasks(2, None) + phase_tasks(3, None)[:1],
                      outproj_units(1) + qkv_units(3), 96)
            # close qkv pools: frees ~72KB/partition for the tail phase
            qkv_scope.close()
            with tc.tile_pool(name="late_p", bufs=6) as lp, \
                 tc.tile_pool(name="late_at", bufs=1) as lat, \
                 tc.tile_pool(name="late_bcs", bufs=4) as lbc:
                late = {"p": lp, "at": lat, "bcs": lbc}
                # phase 3: att(3) pairs 1-3 + op(2)
                run_phase(phase_tasks(3, None)[1:],
                          outproj_units(2), 96)
            late = None
            for u in outproj_units(NQ - 1):
                u()
    nc.finalize()
    return nc


_NC_CACHE = {}


def _get_nc(apply_mask=True):
    key = ("nc", bool(apply_mask))
    if key not in _NC_CACHE:
        _NC_CACHE[key] = build_nc(apply_mask)
    return _NC_CACHE[key]


def _make_in_maps(x, w_qkv, w_out, attn_mask):
    x = np.asarray(x, dtype=np.float32)
    w_qkv = np.asarray(w_qkv, dtype=np.float32)
    w_out = np.asarray(w_out, dtype=np.float32)
    am = np.asarray(attn_mask)
    in_maps = []
    for c in range(NCORES):
        b, hg = c // 2, c % 2
        wqk_c = np.ascontiguousarray(np.concatenate(
            [w_qkv[:, hg * CQ:(hg + 1) * CQ],
             w_qkv[:, DIM + hg * CQ:DIM + (hg + 1) * CQ]], axis=1))
        wv_c = np.ascontiguousarray(w_qkv[:, 2 * DIM + hg * CQ:2 * DIM + (hg + 1) * CQ])
        wo_c = np.ascontiguousarray(w_out[hg * CQ:(hg + 1) * CQ, :])
        mv_c = np.ascontiguousarray(
            am[b].astype(np.float32).reshape(NT, 128).T)
        in_maps.append({
            "x": np.ascontiguousarray(x[b]),
            "wqk": wqk_c,
            "wv": wv_c,
            "wo": wo_c,
            "maskv": mv_c,
        })
    return in_maps


def run(x, w_qkv, w_out, attn_mask, trace=False):
    nc = _get_nc(apply_mask=not np.all(np.asarray(attn_mask)))
    in_maps = _make_in_maps(x, w_qkv, w_out, attn_mask)
    res = run_bass_kernel_spmd(nc, in_maps, list(range(NCORES)), trace=trace)
    outs = [res.results[c]["out"] for c in range(NCORES)]
    full = np.stack([outs[2 * b] + outs[2 * b + 1] for b in range(B)], axis=0)
    return full.astype(np.float32), res


def kernel(x, w_qkv, w_out, attn_mask):
    full, _ = run(x, w_qkv, w_out, attn_mask, trace=False)
    return full



# revision 3
# speedup vs baseline: 1.1515x; 1.1515x over previous
"""Causal multi-head attention block (qkv proj + attention + out proj) on 8
Trainium2 NeuronCores.

Sharding: core c = 2*b + hg handles batch b (of 4) and head-group hg (8 of 16
heads).  Each core computes qkv for its heads, causal attention, and a partial
out-projection (its 512 rows of w_out); the host sums the two head-group
partials per batch.

v1 layout (bf16 operands, fp32 PSUM):
  - x arrives pre-transposed from the host as xT [DIM, T] bf16; per t-quarter
    one DMA loads the 8 [128, 512] contraction chunks (no PE transposes).
  - weights arrive bf16, each as ONE large DMA (split across all 16 SDMA
    engines by the runtime).
  - Q^T/K^T come out of the projection as head-pair tiles [128 = 2 heads x 64,
    t]; V in natural [t, c] layout augmented with a ones column per head
    (V_aug), so P @ V_aug accumulates the numerator and softmax denominator
    together (no max-subtraction: scores ~ N(0,1), exp safe).
  - scores are computed transposed, S^T[k, q], two heads concurrently via PE
    row tiling (K=64 each) into one two-bank PSUM tile; exp (scale fused) is
    one ACT op per pair, narrowed on diagonal blocks; causal masking is a 0/1
    multiply on one 128-wide strip.
  - normalization per pair: the denominator row is copied to SBUF, stored to
    DRAM once, partition-broadcast back (gpsimd/SWDGE queues, off the SP
    critical path), inverted with reciprocal_approx_fast [128, 512], and
    multiplied into the attention tile.
  - emission is phase-interleaved: qkv quarter q+1 and out_proj q-1 are pumped
    into attention block q at iteration granularity; the final out_proj block
    is split m-wise (heads 0-5 accumulated into SBUF early, heads 6-7 joined
    after the last normalization) to shorten the tail.
"""

import sys

if "/opt/trn_rl_repo" not in sys.path:
    sys.path.insert(0, "/opt/trn_rl_repo")

import numpy as np

import concourse.bass as bass
import concourse.mybir as mybir
import concourse.tile as tile
from concourse import bacc
from concourse.bass_utils import run_bass_kernel_spmd

DIM = 1024
N_HEAD = 16
HD = 64
B, T = 4, 2048
HG = 8          # heads per core
CQ = HG * HD    # 512 feature columns per group
NCORES = 8
NT = T // 128   # 16 t-subtiles
NQ = T // 512   # 4 quarters / q-blocks

f32 = mybir.dt.float32
bf16 = mybir.dt.bfloat16
Exp = mybir.ActivationFunctionType.Exp
AluAdd = mybir.AluOpType.add


def build_nc():
    nc = bacc.Bacc(None, target_bir_lowering=False)
    xt_d = nc.declare_dram_parameter("xt", [DIM, T], bf16, isOutput=False)
    wqk_d = nc.declare_dram_parameter("wqk", [DIM, 2 * CQ], bf16, isOutput=False)
    wv_d = nc.declare_dram_parameter("wv", [DIM, CQ], bf16, isOutput=False)
    wo_d = nc.declare_dram_parameter("wo", [CQ, DIM], bf16, isOutput=False)
    mv_d = nc.declare_dram_parameter("maskv", [128, NT], f32, isOutput=False)
    out_d = nc.declare_dram_parameter("out", [T, DIM], f32, isOutput=True)

    with tile.TileContext(nc) as tc:
        with tc.tile_pool(name="pp", bufs=1) as pp, \
             tc.tile_pool(name="xq_p", bufs=2) as xq_p, \
             tc.tile_pool(name="qtp", bufs=2) as qtp, \
             tc.tile_pool(name="p_p", bufs=4) as p_p, \
             tc.tile_pool(name="at_p", bufs=2) as at_p, \
             tc.tile_pool(name="den_p", bufs=2) as den_p, \
             tc.tile_pool(name="bcs_p", bufs=2) as bcs_p, \
             tc.tile_pool(name="acc_p", bufs=1) as acc_p, \
             tc.tile_pool(name="out_p", bufs=2) as out_p, \
             tc.tile_pool(name="dram_p", bufs=2, space="DRAM") as dram_p, \
             tc.tile_pool(name="ps_aux", bufs=2, space="PSUM") as ps_aux, \
             tc.tile_pool(name="ps_s", bufs=2, space="PSUM") as ps_s, \
             tc.tile_pool(name="ps_pv", bufs=1, space="PSUM") as ps_pv:

            # ---- constants ----
            mv_sb = pp.tile([128, NT], f32, name="maskv_sb", tag="maskv_sb")
            nc.sync.dma_start(out=mv_sb, in_=mv_d[:, :])
            # one 128x128 causal strip: keep where q_local >= k_local
            dstrip = pp.tile([128, 128], bf16, name="dstrip", tag="dstrip")
            nc.gpsimd.memset(dstrip, 1.0)
            nc.gpsimd.affine_select(
                out=dstrip, in_=dstrip, compare_op=mybir.AluOpType.is_ge,
                fill=0.0, base=0, pattern=[[1, 128]], channel_multiplier=-1)
            onescol = pp.tile([128, HG], bf16, name="onescol", tag="onescol")
            nc.vector.memset(onescol, 1.0)

            # ---- persistent weights: one big DMA each ----
            wqk_sb = pp.tile([128, 8, 2 * CQ], bf16, name="wqk_sb", tag="wqk_sb")
            wv_sb = pp.tile([128, 8, CQ], bf16, name="wv_sb", tag="wv_sb")
            wo_sb = pp.tile([128, 4, DIM], bf16, name="wo_sb", tag="wo_sb")
            # x quarter tiles [128, 8 kb chunks, 512]; first quarter loads now
            xts_cur = {}

            def load_x_quarter(q):
                xq = xq_p.tile([128, 8, 512], bf16, name=f"xq{q}", tag="xq")
                nc.sync.dma_start(
                    out=xq,
                    in_=xt_d[:, q * 512:(q + 1) * 512]
                    .rearrange("(kb p) t -> p kb t", p=128))
                xts_cur[q] = xq

            load_x_quarter(0)
            nc.sync.dma_start(
                out=wqk_sb, in_=wqk_d.rearrange("(kb p) n -> p kb n", p=128))
            nc.sync.dma_start(
                out=wv_sb, in_=wv_d.rearrange("(kb p) n -> p kb n", p=128))
            nc.sync.dma_start(
                out=wo_sb, in_=wo_d.rearrange("(m p) n -> p m n", p=128))

            # ---- persistent tensors ----
            kt = [pp.tile([128, T], bf16, name=f"kt{m}", tag=f"kt{m}") for m in range(4)]
            vaug = [pp.tile([128, HG * 65], bf16, name=f"vaug{t}", tag=f"vaug{t}")
                    for t in range(NT)]

            qt_cur = {}    # quarter -> [4 pair tiles [128, 512]]
            ats_cur = {}   # qb -> [4 pair tiles [128, 512]]

            # ---------- qkv quarter units ----------
            def qkv_units(q, with_xload=None):
                units = []
                if with_xload is not None:
                    units.append(lambda: load_x_quarter(with_xload))
                qt_cur[q] = [None] * 4

                def qk_unit(m):
                    xq = xts_cur[q]
                    pq = ps_aux.tile([128, 512], f32, name="mm", tag="aux")
                    for kb in range(8):
                        nc.tensor.matmul(
                            pq, wqk_sb[:, kb, m * 128:(m + 1) * 128], xq[:, kb, :],
                            start=(kb == 0), stop=(kb == 7))
                    if m < 4:
                        qtile = qtp.tile([128, 512], bf16, name=f"qt{m}", tag=f"qt{m}")
                        nc.vector.tensor_copy(qtile, pq)
                        qt_cur[q][m] = qtile
                    else:
                        nc.vector.tensor_copy(
                            kt[m - 4][:, q * 512:(q + 1) * 512], pq)
                for m in range(8):
                    units.append(lambda m=m: qk_unit(m))

                def v_unit(ti):
                    xq = xts_cur[q]
                    pv = ps_aux.tile([128, 512], f32, name="mm", tag="aux")
                    for kb in range(8):
                        nc.tensor.matmul(
                            pv, xq[:, kb, ti * 128:(ti + 1) * 128], wv_sb[:, kb, :],
                            start=(kb == 0), stop=(kb == 7))
                    vt = vaug[q * 4 + ti]
                    vt3 = vt.rearrange("p (h w) -> p h w", w=65)
                    nc.vector.tensor_copy(
                        vt3[:, :, 0:64], pv.rearrange("p (h w) -> p h w", w=64))
                    nc.vector.tensor_copy(
                        vt3[:, :, 64:65], onescol.rearrange("p (h w) -> p h w", w=1))
                    nc.vector.tensor_scalar_mul(
                        vt, vt, mv_sb[:, (q * 4 + ti):(q * 4 + ti + 1)])
                for ti in range(4):
                    units.append(lambda ti=ti: v_unit(ti))
                return units

            # ---------- out_proj units ----------
            ob_cur = {}

            def op_mm(po, qb, m, ti, nb, start, stop):
                nc.tensor.matmul(
                    po, ats_cur[qb][m][:, ti * 128:(ti + 1) * 128],
                    wo_sb[:, m, nb * 512:(nb + 1) * 512],
                    start=start, stop=stop)

            def op_store(qb, ti, ob):
                t0 = (qb * 4 + ti) * 128
                nc.sync.dma_start(out=out_d[t0:t0 + 128, :], in_=ob)

            def outproj_units(qb):
                units = []

                def op_unit(ti, nb):
                    po = ps_aux.tile([128, 512], f32, name="mm", tag="aux")
                    for m in range(4):
                        op_mm(po, qb, m, ti, nb, start=(m == 0), stop=(m == 3))
                    if nb == 0:
                        ob = out_p.tile([128, DIM], f32, name="ob", tag="ob")
                        ob_cur[qb, ti] = ob
                    ob = ob_cur[qb, ti]
                    nc.vector.tensor_copy(ob[:, nb * 512:(nb + 1) * 512], po)
                    if nb == 1:
                        op_store(qb, ti, ob)
                for ti in range(4):
                    for nb in range(2):
                        units.append(lambda ti=ti, nb=nb: op_unit(ti, nb))
                return units

            # last q-block: heads 0-5 (m=0..2) accumulated into SBUF before the
            # final pair normalizes; m=3 joins after.
            acc_sb = {}

            def op_partial_units(qb):
                units = []

                def part_unit(ti, nb):
                    po = ps_aux.tile([128, 512], f32, name="mm", tag="aux")
                    for m in range(3):
                        op_mm(po, qb, m, ti, nb, start=(m == 0), stop=(m == 2))
                    acc = acc_p.tile([128, 512], f32, name=f"acc{ti}{nb}",
                                     tag=f"acc{ti}{nb}")
                    nc.vector.tensor_copy(acc, po)
                    acc_sb[ti, nb] = acc
                for ti in range(4):
                    for nb in range(2):
                        units.append(lambda ti=ti, nb=nb: part_unit(ti, nb))
                return units

            def op_final_units(qb):
                units = []

                def fin_unit(ti, nb):
                    po = ps_aux.tile([128, 512], f32, name="mm", tag="aux")
                    op_mm(po, qb, 3, ti, nb, start=True, stop=True)
                    if nb == 0:
                        ob = out_p.tile([128, DIM], f32, name="ob", tag="ob")
                        ob_cur[qb, ti] = ob
                    ob = ob_cur[qb, ti]
                    nc.vector.tensor_tensor(
                        ob[:, nb * 512:(nb + 1) * 512], po, acc_sb[ti, nb], AluAdd)
                    if nb == 1:
                        op_store(qb, ti, ob)
                for ti in range(4):
                    for nb in range(2):
                        units.append(lambda ti=ti, nb=nb: fin_unit(ti, nb))
                return units

            # ---------- attention pair ----------
            def att_pair(qb, m, pump):
                nk = 4 * (qb + 1)
                pvp = ps_pv.tile([65, 1024], f32, name="pv", tag="pv")

                def pv_mms(pk, pt, w0, stop):
                    # masked q-columns [0:w0) of this k-tile are exactly zero:
                    # skip them; PSUM accumulation keeps their prior value
                    nc.tensor.matmul(
                        pvp[:, w0:512],
                        vaug[pk][:, (2 * m) * 65:(2 * m + 1) * 65],
                        pt[:, w0:512], start=(pk == 0), stop=stop)
                    nc.tensor.matmul(
                        pvp[:, 512 + w0:1024],
                        vaug[pk][:, (2 * m + 1) * 65:(2 * m + 2) * 65],
                        pt[:, 512 + w0:1024], start=(pk == 0), stop=stop)

                prev = None
                for kti in range(nk):
                    j = kti - 4 * qb
                    w0 = 128 * j if j > 0 else 0
                    sp = ps_s.tile([128, 1024], f32, name="s", tag="s")
                    nc.tensor.matmul(
                        sp[:, w0:512],
                        kt[m][0:64, kti * 128:(kti + 1) * 128],
                        qt_cur[qb][m][0:64, w0:512], start=True, stop=True)
                    nc.tensor.matmul(
                        sp[:, 512 + w0:1024],
                        kt[m][64:128, kti * 128:(kti + 1) * 128],
                        qt_cur[qb][m][64:128, w0:512], start=True, stop=True)
                    pump()
                    ppt = p_p.tile([128, 1024], bf16, name="p", tag="p")
                    p3 = ppt.rearrange("p (h w) -> p h w", w=512)
                    s3 = sp.rearrange("p (h w) -> p h w", w=512)
                    if j < 0:
                        nc.scalar.activation(p3, s3, Exp, scale=0.125)
                    else:
                        nc.scalar.activation(
                            p3[:, :, w0:512], s3[:, :, w0:512], Exp, scale=0.125)
                        for h in range(2):
                            nc.vector.tensor_mul(
                                ppt[:, h * 512 + w0:h * 512 + w0 + 128],
                                ppt[:, h * 512 + w0:h * 512 + w0 + 128],
                                dstrip)
                    if prev is not None:
                        pv_mms(*prev, stop=False)
                    prev = (kti, ppt, w0)
                    pump()
                pv_mms(*prev, stop=True)

                atm = at_p.tile([128, 512], bf16, name=f"at{m}", tag=f"at{m}")
                ats_cur[qb][m] = atm
                nc.vector.tensor_copy(atm[0:64, :], pvp[0:64, 0:512])
                nc.vector.tensor_copy(atm[64:128, :], pvp[0:64, 512:1024])
                # denominator: SBUF -> DRAM -> partition-broadcast back, then
                # fast reciprocal at 128 lanes; DMAs ride SWDGE (gpsimd) queues
                dn = den_p.tile([1, 1024], f32, name="dn", tag="dn")
                nc.vector.tensor_copy(dn, pvp[64:65, :])
                d1 = dram_p.tile([1, 1024], f32, name="d1", tag="d1")
                nc.gpsimd.dma_start(out=d1, in_=dn)
                bcs = bcs_p.tile([128, 512], f32, name="bcs", tag="bcs")
                for h in range(2):
                    nc.gpsimd.dma_start(
                        out=bcs[h * 64:(h + 1) * 64, :],
                        in_=d1[0:1, h * 512:(h + 1) * 512].partition_broadcast(64))
                nc.vector.reciprocal_approx_fast(out=bcs, in_=bcs)
                bcsb = bcs_p.tile([128, 512], bf16, name="bcsb", tag="bcsb")
                nc.vector.tensor_copy(bcsb, bcs)
                nc.vector.tensor_mul(atm, atm, bcsb)

            def run_phase(tasks, fillers, n_units):
                """tasks: closures taking pump(); fillers pumped proportionally."""
                nf = len(fillers)
                state = {"fi": 0, "ai": 0}

                def pump():
                    state["ai"] += 1
                    while state["fi"] * n_units < state["ai"] * nf \
                            and state["fi"] < nf:
                        fillers[state["fi"]]()
                        state["fi"] += 1
                for t in tasks:
                    t(pump)
                while state["fi"] < nf:
                    fillers[state["fi"]]()
                    state["fi"] += 1

            # ---------------- emission schedule ----------------
            for u in qkv_units(0):
                u()
            for qb in range(NQ):
                ats_cur[qb] = [None] * 4

            def phase_tasks(qb, ms):
                def mk(m):
                    def t(pump):
                        att_pair(qb, m, pump)
                    return t
                return [mk(m) for m in ms]

            # phase 0: att(0) + qkv(1)
            run_phase(phase_tasks(0, range(4)),
                      qkv_units(1, with_xload=1), 32)
            # phase 1: att(1) + op(0) + qkv(2)
            run_phase(phase_tasks(1, range(4)),
                      outproj_units(0) + qkv_units(2, with_xload=2), 64)
            # phase 2: att(2) + op(1) + qkv(3)
            run_phase(phase_tasks(2, range(4)),
                      outproj_units(1) + qkv_units(3, with_xload=3), 96)
            # phase 3: att(3) pairs 0-2 + op(2)
            run_phase(phase_tasks(3, range(3)),
                      outproj_units(2), 72)
            # phase 4: att(3) pair 3 + partial out_proj over pairs 0-2
            run_phase(phase_tasks(3, [3]),
                      op_partial_units(3), 32)
            # tail: only the m=3 out_proj slices depend on the last pair
            for u in op_final_units(3):
                u()
    nc.finalize()
    return nc


_NC_CACHE = {}


def _get_nc():
    if "nc" not in _NC_CACHE:
        _NC_CACHE["nc"] = build_nc()
    return _NC_CACHE["nc"]


def _make_in_maps(x, w_qkv, w_out, attn_mask):
    np_bf16 = mybir.dt.np(bf16)
    x = np.asarray(x, dtype=np.float32)
    w_qkv = np.asarray(w_qkv, dtype=np.float32)
    w_out = np.asarray(w_out, dtype=np.float32)
    am = np.asarray(attn_mask)
    in_maps = []
    for c in range(NCORES):
        b, hg = c // 2, c % 2
        wqk_c = np.ascontiguousarray(np.concatenate(
            [w_qkv[:, hg * CQ:(hg + 1) * CQ],
             w_qkv[:, DIM + hg * CQ:DIM + (hg + 1) * CQ]], axis=1)).astype(np_bf16)
        wv_c = np.ascontiguousarray(
            w_qkv[:, 2 * DIM + hg * CQ:2 * DIM + (hg + 1) * CQ]).astype(np_bf16)
        wo_c = np.ascontiguousarray(w_out[hg * CQ:(hg + 1) * CQ, :]).astype(np_bf16)
        mv_c = np.ascontiguousarray(
            am[b].astype(np.float32).reshape(NT, 128).T)
        xt_c = np.ascontiguousarray(x[b].T).astype(np_bf16)
        in_maps.append({
            "xt": xt_c,
            "wqk": wqk_c,
            "wv": wv_c,
            "wo": wo_c,
            "maskv": mv_c,
        })
    return in_maps


def run(x, w_qkv, w_out, attn_mask, trace=False):
    nc = _get_nc()
    in_maps = _make_in_maps(x, w_qkv, w_out, attn_mask)
    res = run_bass_kernel_spmd(nc, in_maps, list(range(NCORES)), trace=trace)
    outs = [res.results[c]["out"] for c in range(NCORES)]
    full = np.stack([outs[2 * b] + outs[2 * b + 1] for b in range(B)], axis=0)
    return full.astype(np.float32), res


def kernel(x, w_qkv, w_out, attn_mask):
    full, _ = run(x, w_qkv, w_out, attn_mask, trace=False)
    return full


# revision 5
# speedup vs baseline: 1.1737x; 1.0193x over previous
"""Causal multi-head attention block (qkv proj + attention + out proj) on 8
Trainium2 NeuronCores.

Sharding: core c = 2*b + hg handles batch b (of 4) and head-group hg (8 of 16
heads).  Each core computes qkv for its heads, causal attention, and a partial
out-projection (its 512 rows of w_out); the host sums the two head-group
partials per batch.

v1 layout (bf16 operands, fp32 PSUM):
  - x arrives pre-transposed from the host as xT [DIM, T] bf16; per t-quarter
    one DMA loads the 8 [128, 512] contraction chunks (no PE transposes).
  - weights arrive bf16, each as ONE large DMA (split across all 16 SDMA
    engines by the runtime).
  - Q^T/K^T come out of the projection as head-pair tiles [128 = 2 heads x 64,
    t]; V in natural [t, c] layout augmented with a ones column per head
    (V_aug), so P @ V_aug accumulates the numerator and softmax denominator
    together (no max-subtraction: scores ~ N(0,1), exp safe).
  - scores are computed transposed, S^T[k, q], two heads concurrently via PE
    row tiling (K=64 each) into one two-bank PSUM tile; exp (scale fused) is
    one ACT op per pair, narrowed on diagonal blocks; causal masking is a 0/1
    multiply on one 128-wide strip.
  - normalization per pair: the denominator row is copied to SBUF, stored to
    DRAM once, partition-broadcast back (gpsimd/SWDGE queues, off the SP
    critical path), inverted with reciprocal_approx_fast [128, 512], and
    multiplied into the attention tile.
  - emission is phase-interleaved: qkv quarter q+1 and out_proj q-1 are pumped
    into attention block q at iteration granularity; the final out_proj block
    is split m-wise (heads 0-5 accumulated into SBUF early, heads 6-7 joined
    after the last normalization) to shorten the tail.
"""

import sys

if "/opt/trn_rl_repo" not in sys.path:
    sys.path.insert(0, "/opt/trn_rl_repo")

import numpy as np

import concourse.bass as bass
import concourse.mybir as mybir
import concourse.tile as tile
from concourse import bacc
from concourse.bass_utils import run_bass_kernel_spmd

DIM = 1024
N_HEAD = 16
HD = 64
B, T = 4, 2048
HG = 8          # heads per core
CQ = HG * HD    # 512 feature columns per group
NCORES = 8
NT = T // 128   # 16 t-subtiles
NQ = T // 512   # 4 quarters / q-blocks

f32 = mybir.dt.float32
f32r = mybir.dt.float32r
bf16 = mybir.dt.bfloat16
Exp = mybir.ActivationFunctionType.Exp
AluAdd = mybir.AluOpType.add


def build_nc():
    nc = bacc.Bacc(None, target_bir_lowering=False)
    xt_d = nc.declare_dram_parameter("xt", [DIM, T], bf16, isOutput=False)
    wqk_d = nc.declare_dram_parameter("wqk", [DIM, 2 * CQ], bf16, isOutput=False)
    wv_d = nc.declare_dram_parameter("wv", [DIM, CQ], bf16, isOutput=False)
    wo_d = nc.declare_dram_parameter("wo", [CQ, DIM], bf16, isOutput=False)
    mv_d = nc.declare_dram_parameter("maskv", [128, NT], f32, isOutput=False)
    out_d = nc.declare_dram_parameter("out", [T, DIM], f32, isOutput=True)

    with tile.TileContext(nc) as tc:
        with tc.tile_pool(name="pp", bufs=1) as pp, \
             tc.tile_pool(name="xq_p", bufs=2) as xq_p, \
             tc.tile_pool(name="qtp", bufs=2) as qtp, \
             tc.tile_pool(name="p_p", bufs=6) as p_p, \
             tc.tile_pool(name="at_p", bufs=2) as at_p, \
             tc.tile_pool(name="den_p", bufs=2) as den_p, \
             tc.tile_pool(name="bcs_p", bufs=2) as bcs_p, \
             tc.tile_pool(name="acc_p", bufs=1) as acc_p, \
             tc.tile_pool(name="out_p", bufs=2) as out_p, \
             tc.tile_pool(name="ps_aux", bufs=2, space="PSUM") as ps_aux, \
             tc.tile_pool(name="ps_s", bufs=2, space="PSUM") as ps_s, \
             tc.tile_pool(name="ps_pv", bufs=1, space="PSUM") as ps_pv:

            # ---- constants ----
            mv_sb = pp.tile([128, NT], f32, name="maskv_sb", tag="maskv_sb")
            nc.scalar.dma_start(out=mv_sb, in_=mv_d[:, :])
            # one 128x128 causal strip: keep where q_local >= k_local
            dstrip = pp.tile([128, 128], f32, name="dstrip", tag="dstrip")
            nc.gpsimd.memset(dstrip, 1.0)
            nc.gpsimd.affine_select(
                out=dstrip, in_=dstrip, compare_op=mybir.AluOpType.is_ge,
                fill=0.0, base=0, pattern=[[1, 128]], channel_multiplier=-1)
            onescol = pp.tile([128, HG], f32, name="onescol", tag="onescol")
            nc.vector.memset(onescol, 1.0)
            # head-half selector rows for the PE denominator broadcast:
            # bmh[0, 0:128] selects partitions 0-63, bmh[0, 128:256] selects
            # partitions 64-127 (K=1 matmuls: out[p, q] = bmh[p] * den[q])
            bmh = pp.tile([1, 256], bf16, name="bmh", tag="bmh")
            nc.vector.memset(bmh, 0.0)
            nc.vector.memset(bmh[0:1, 0:64], 1.0)
            nc.vector.memset(bmh[0:1, 192:256], 1.0)

            # ---- persistent weights: one big DMA each ----
            wqk_sb = pp.tile([128, 8, 2 * CQ], bf16, name="wqk_sb", tag="wqk_sb")
            wv_sb = pp.tile([128, 8, CQ], bf16, name="wv_sb", tag="wv_sb")
            wo_sb = pp.tile([128, 4, DIM], bf16, name="wo_sb", tag="wo_sb")
            # x quarter tiles [128, 8 kb chunks, 512]; first quarter loads now
            xts_cur = {}

            def load_x_quarter(q):
                xq = xq_p.tile([128, 8, 512], bf16, name=f"xq{q}", tag="xq")
                x3 = xt_d[:, q * 512:(q + 1) * 512].rearrange(
                    "(kb p) t -> p kb t", p=128)
                nc.sync.dma_start(out=xq[:, 0:4, :], in_=x3[:, 0:4, :])
                nc.scalar.dma_start(out=xq[:, 4:8, :], in_=x3[:, 4:8, :])
                xts_cur[q] = xq

            wqk3 = wqk_d.rearrange("(kb p) n -> p kb n", p=128)
            nc.sync.dma_start(out=wqk_sb[:, 0:4, :], in_=wqk3[:, 0:4, :])
            nc.scalar.dma_start(out=wqk_sb[:, 4:8, :], in_=wqk3[:, 4:8, :])
            load_x_quarter(0)
            nc.scalar.dma_start(
                out=wv_sb, in_=wv_d.rearrange("(kb p) n -> p kb n", p=128))
            nc.sync.dma_start(
                out=wo_sb, in_=wo_d.rearrange("(m p) n -> p m n", p=128))

            # ---- persistent tensors ----
            kt = [pp.tile([128, T], bf16, name=f"kt{m}", tag=f"kt{m}") for m in range(4)]
            vaug = [pp.tile([128, HG * 65], f32r, name=f"vaug{t}", tag=f"vaug{t}")
                    for t in range(NT)]

            qt_cur = {}    # quarter -> [4 pair tiles [128, 512]]
            ats_cur = {}   # qb -> [4 pair tiles [128, 512]]

            # ---------- qkv quarter units ----------
            def qkv_units(q, with_xload=None):
                units = []
                if with_xload is not None:
                    units.append(lambda: load_x_quarter(with_xload))
                qt_cur[q] = [None] * 4

                def qk_unit(m):
                    xq = xts_cur[q]
                    pq = ps_aux.tile([128, 512], f32, name="mm", tag="aux")
                    for kb in range(8):
                        nc.tensor.matmul(
                            pq, wqk_sb[:, kb, m * 128:(m + 1) * 128], xq[:, kb, :],
                            start=(kb == 0), stop=(kb == 7))
                    if m < 4:
                        qtile = qtp.tile([128, 512], bf16, name=f"qt{m}", tag=f"qt{m}")
                        nc.vector.tensor_copy(qtile, pq)
                        qt_cur[q][m] = qtile
                    else:
                        nc.vector.tensor_copy(
                            kt[m - 4][:, q * 512:(q + 1) * 512], pq)
                for m in range(8):
                    units.append(lambda m=m: qk_unit(m))

                def v_unit(ti):
                    xq = xts_cur[q]
                    pv = ps_aux.tile([128, 512], f32, name="mm", tag="aux")
                    for kb in range(8):
                        nc.tensor.matmul(
                            pv, xq[:, kb, ti * 128:(ti + 1) * 128], wv_sb[:, kb, :],
                            start=(kb == 0), stop=(kb == 7))
                    vt = vaug[q * 4 + ti]
                    vt3 = vt.rearrange("p (h w) -> p h w", w=65)
                    nc.vector.tensor_copy(
                        vt3[:, :, 0:64], pv.rearrange("p (h w) -> p h w", w=64))
                    nc.vector.tensor_copy(
                        vt3[:, :, 64:65], onescol.rearrange("p (h w) -> p h w", w=1))
                    nc.vector.tensor_scalar_mul(
                        vt, vt, mv_sb[:, (q * 4 + ti):(q * 4 + ti + 1)])
                for ti in range(4):
                    units.append(lambda ti=ti: v_unit(ti))
                return units

            # ---------- out_proj units ----------
            ob_cur = {}

            def op_mm(po, qb, m, ti, nb, start, stop):
                nc.tensor.matmul(
                    po, ats_cur[qb][m][:, ti * 128:(ti + 1) * 128],
                    wo_sb[:, m, nb * 512:(nb + 1) * 512],
                    start=start, stop=stop)

            def op_store(qb, ti, ob):
                t0 = (qb * 4 + ti) * 128
                nc.sync.dma_start(out=out_d[t0:t0 + 128, :], in_=ob)

            def outproj_units(qb):
                units = []

                def op_unit(ti, nb):
                    po = ps_aux.tile([128, 512], f32, name="mm", tag="aux")
                    for m in range(4):
                        op_mm(po, qb, m, ti, nb, start=(m == 0), stop=(m == 3))
                    if nb == 0:
                        ob = out_p.tile([128, DIM], f32, name="ob", tag="ob")
                        ob_cur[qb, ti] = ob
                    ob = ob_cur[qb, ti]
                    nc.vector.tensor_copy(ob[:, nb * 512:(nb + 1) * 512], po)
                    if nb == 1:
                        op_store(qb, ti, ob)
                for ti in range(4):
                    for nb in range(2):
                        units.append(lambda ti=ti, nb=nb: op_unit(ti, nb))
                return units

            # last q-block: heads 0-5 (m=0..2) accumulated into SBUF before the
            # final pair normalizes; m=3 joins after.
            acc_sb = {}

            def op_partial_units(qb):
                units = []

                def part_unit(ti, nb):
                    po = ps_aux.tile([128, 512], f32, name="mm", tag="aux")
                    for m in range(3):
                        op_mm(po, qb, m, ti, nb, start=(m == 0), stop=(m == 2))
                    acc = acc_p.tile([128, 512], f32, name=f"acc{ti}{nb}",
                                     tag=f"acc{ti}{nb}")
                    nc.vector.tensor_copy(acc, po)
                    acc_sb[ti, nb] = acc
                for ti in range(4):
                    for nb in range(2):
                        units.append(lambda ti=ti, nb=nb: part_unit(ti, nb))
                return units

            def op_final_units(qb):
                units = []

                def fin_unit(ti, nb):
                    po = ps_aux.tile([128, 512], f32, name="mm", tag="aux")
                    op_mm(po, qb, 3, ti, nb, start=True, stop=True)
                    if nb == 0:
                        ob = out_p.tile([128, DIM], f32, name="ob", tag="ob")
                        ob_cur[qb, ti] = ob
                    ob = ob_cur[qb, ti]
                    nc.vector.tensor_tensor(
                        ob[:, nb * 512:(nb + 1) * 512], po, acc_sb[ti, nb], AluAdd)
                    if nb == 1:
                        op_store(qb, ti, ob)
                for ti in range(4):
                    for nb in range(2):
                        units.append(lambda ti=ti, nb=nb: fin_unit(ti, nb))
                return units

            # ---------- attention pair ----------
            def att_pair(qb, m, pump):
                nk = 4 * (qb + 1)
                pvp = ps_pv.tile([65, 1024], f32, name="pv", tag="pv")

                def pv_mms(pk, pt, w0, stop):
                    # masked q-columns [0:w0) of this k-tile are exactly zero:
                    # skip them; PSUM accumulation keeps their prior value
                    nc.tensor.matmul(
                        pvp[:, w0:512],
                        vaug[pk][:, (2 * m) * 65:(2 * m + 1) * 65],
                        pt[:, w0:512], start=(pk == 0), stop=stop)
                    nc.tensor.matmul(
                        pvp[:, 512 + w0:1024],
                        vaug[pk][:, (2 * m + 1) * 65:(2 * m + 2) * 65],
                        pt[:, 512 + w0:1024], start=(pk == 0), stop=stop)

                prev = None
                for kti in range(nk):
                    j = kti - 4 * qb
                    w0 = 128 * j if j > 0 else 0
                    sp = ps_s.tile([128, 1024], f32, name="s", tag="s")
                    nc.tensor.matmul(
                        sp[:, w0:512],
                        kt[m][0:64, kti * 128:(kti + 1) * 128],
                        qt_cur[qb][m][0:64, w0:512], start=True, stop=True)
                    nc.tensor.matmul(
                        sp[:, 512 + w0:1024],
                        kt[m][64:128, kti * 128:(kti + 1) * 128],
                        qt_cur[qb][m][64:128, w0:512], start=True, stop=True)
                    pump()
                    ppt = p_p.tile([128, 1024], f32r, name="p", tag="p")
                    p3 = ppt.rearrange("p (h w) -> p h w", w=512)
                    s3 = sp.rearrange("p (h w) -> p h w", w=512)
                    if j < 0:
                        nc.scalar.activation(p3, s3, Exp, scale=0.125)
                    else:
                        nc.scalar.activation(
                            p3[:, :, w0:512], s3[:, :, w0:512], Exp, scale=0.125)
                        for h in range(2):
                            nc.gpsimd.tensor_mul(
                                ppt[:, h * 512 + w0:h * 512 + w0 + 128],
                                ppt[:, h * 512 + w0:h * 512 + w0 + 128],
                                dstrip)
                    if prev is not None:
                        pv_mms(*prev, stop=False)
                    prev = (kti, ppt, w0)
                    pump()
                pv_mms(*prev, stop=True)

                atm = at_p.tile([128, 512], bf16, name=f"at{m}", tag=f"at{m}")
                ats_cur[qb][m] = atm
                # the last pair's chain is the kernel tail: route its copies
                # through the (then idle) scalar engine to cut latency
                cp = nc.scalar.copy if (qb == NQ - 1 and m == 3) else \
                    nc.vector.tensor_copy
                cp(atm[0:64, :], pvp[0:64, 0:512])
                cp(atm[64:128, :], pvp[0:64, 512:1024])
                # denominator: partition-broadcast via two K=1 PE matmuls
                # (bmh half ^T @ den half), then reciprocal at 128 lanes
                denb = den_p.tile([1, 1024], bf16, name="denb", tag="denb")
                cp(denb, pvp[64:65, :])
                bfull = ps_s.tile([128, 1024], f32, name="s", tag="s")
                for h in range(2):
                    nc.tensor.matmul(
                        bfull[:, 0:512], bmh[0:1, h * 128:(h + 1) * 128],
                        denb[0:1, h * 512:(h + 1) * 512],
                        start=(h == 0), stop=(h == 1))
                bcs = bcs_p.tile([128, 512], f32, name="bcs", tag="bcs")
                nc.vector.reciprocal_approx_fast(out=bcs, in_=bfull[:, 0:512])
                bcsb = bcs_p.tile([128, 512], bf16, name="bcsb", tag="bcsb")
                cp(bcsb, bcs)
                nc.vector.tensor_mul(atm, atm, bcsb)

            def run_phase(tasks, fillers, n_units):
                """tasks: closures taking pump(); fillers pumped proportionally."""
                nf = len(fillers)
                state = {"fi": 0, "ai": 0}

                def pump():
                    state["ai"] += 1
                    while state["fi"] * n_units < state["ai"] * nf \
                            and state["fi"] < nf:
                        fillers[state["fi"]]()
                        state["fi"] += 1
                for t in tasks:
                    t(pump)
                while state["fi"] < nf:
                    fillers[state["fi"]]()
                    state["fi"] += 1

            # ---------------- emission schedule ----------------
            for u in qkv_units(0):
                u()
            for qb in range(NQ):
                ats_cur[qb] = [None] * 4

            def phase_tasks(qb, ms):
                def mk(m):
                    def t(pump):
                        att_pair(qb, m, pump)
                    return t
                return [mk(m) for m in ms]

            # phase 0: att(0) + qkv(1)
            run_phase(phase_tasks(0, range(4)),
                      qkv_units(1, with_xload=1), 32)
            # phase 1: att(1) + op(0) + qkv(2)
            run_phase(phase_tasks(1, range(4)),
                      outproj_units(0) + qkv_units(2, with_xload=2), 64)
            # phase 2: att(2) + op(1) + qkv(3)
            run_phase(phase_tasks(2, range(4)),
                      outproj_units(1) + qkv_units(3, with_xload=3), 96)
            # phase 3: att(3) pairs 0-2 + op(2)
            run_phase(phase_tasks(3, range(3)),
                      outproj_units(2), 72)
            # phase 4: att(3) pair 3 + partial out_proj over pairs 0-2
            run_phase(phase_tasks(3, [3]),
                      op_partial_units(3), 32)
            # tail: only the m=3 out_proj slices depend on the last pair
            for u in op_final_units(3):
                u()
    nc.finalize()
    return nc


_NC_CACHE = {}


def _get_nc():
    if "nc" not in _NC_CACHE:
        _NC_CACHE["nc"] = build_nc()
    return _NC_CACHE["nc"]


def _make_in_maps(x, w_qkv, w_out, attn_mask):
    np_bf16 = mybir.dt.np(bf16)
    x = np.asarray(x, dtype=np.float32)
    w_qkv = np.asarray(w_qkv, dtype=np.float32)
    w_out = np.asarray(w_out, dtype=np.float32)
    am = np.asarray(attn_mask)
    in_maps = []
    for c in range(NCORES):
        b, hg = c // 2, c % 2
        wqk_c = np.ascontiguousarray(np.concatenate(
            [w_qkv[:, hg * CQ:(hg + 1) * CQ],
             w_qkv[:, DIM + hg * CQ:DIM + (hg + 1) * CQ]], axis=1)).astype(np_bf16)
        wv_c = np.ascontiguousarray(
            w_qkv[:, 2 * DIM + hg * CQ:2 * DIM + (hg + 1) * CQ]).astype(np_bf16)
        wo_c = np.ascontiguousarray(w_out[hg * CQ:(hg + 1) * CQ, :]).astype(np_bf16)
        mv_c = np.ascontiguousarray(
            am[b].astype(np.float32).reshape(NT, 128).T)
        xt_c = np.ascontiguousarray(x[b].T).astype(np_bf16)
        in_maps.append({
            "xt": xt_c,
            "wqk": wqk_c,
            "wv": wv_c,
            "wo": wo_c,
            "maskv": mv_c,
        })
    return in_maps


def run(x, w_qkv, w_out, attn_mask, trace=False):
    nc = _get_nc()
    in_maps = _make_in_maps(x, w_qkv, w_out, attn_mask)
    res = run_bass_kernel_spmd(nc, in_maps, list(range(NCORES)), trace=trace)
    outs = [res.results[c]["out"] for c in range(NCORES)]
    full = np.stack([outs[2 * b] + outs[2 * b + 1] for b in range(B)], axis=0)
    return full.astype(np.float32), res


def kernel(x, w_qkv, w_out, attn_mask):
    full, _ = run(x, w_qkv, w_out, attn_mask, trace=False)
    return full


# revision 7
# speedup vs baseline: 1.2874x; 1.0969x over previous
"""Causal multi-head attention block (qkv proj + attention + out proj) on 8
Trainium2 NeuronCores.

Sharding: core c = 2*b + hg handles batch b (of 4) and head-group hg (8 of 16
heads).  Each core computes qkv for its heads, causal attention, and a partial
out-projection (its 512 rows of w_out); the host sums the two head-group
partials per batch.

v1 layout (bf16 operands, fp32 PSUM):
  - x arrives pre-transposed from the host as xT [DIM, T] bf16; per t-quarter
    one DMA loads the 8 [128, 512] contraction chunks (no PE transposes).
  - weights arrive bf16, each as ONE large DMA (split across all 16 SDMA
    engines by the runtime).
  - Q^T/K^T come out of the projection as head-pair tiles [128 = 2 heads x 64,
    t]; V in natural [t, c] layout augmented with a ones column per head
    (V_aug), so P @ V_aug accumulates the numerator and softmax denominator
    together (no max-subtraction: scores ~ N(0,1), exp safe).
  - scores are computed transposed, S^T[k, q], two heads concurrently via PE
    row tiling (K=64 each) into one two-bank PSUM tile; exp (scale fused) is
    one ACT op per pair, narrowed on diagonal blocks; causal masking is a 0/1
    multiply on one 128-wide strip.
  - normalization per pair: the denominator row is copied to SBUF, stored to
    DRAM once, partition-broadcast back (gpsimd/SWDGE queues, off the SP
    critical path), inverted with reciprocal_approx_fast [128, 512], and
    multiplied into the attention tile.
  - emission is phase-interleaved: qkv quarter q+1 and out_proj q-1 are pumped
    into attention block q at iteration granularity; the final out_proj block
    is split m-wise (heads 0-5 accumulated into SBUF early, heads 6-7 joined
    after the last normalization) to shorten the tail.
"""

import sys

if "/opt/trn_rl_repo" not in sys.path:
    sys.path.insert(0, "/opt/trn_rl_repo")

import numpy as np

import concourse.bass as bass
import concourse.mybir as mybir
import concourse.tile as tile
from concourse import bacc
from concourse.bass_utils import run_bass_kernel_spmd

DIM = 1024
N_HEAD = 16
HD = 64
B, T = 4, 2048
HG = 8          # heads per core
CQ = HG * HD    # 512 feature columns per group
NCORES = 8
NT = T // 128   # 16 t-subtiles
NQ = T // 512   # 4 quarters / q-blocks

f32 = mybir.dt.float32
f32r = mybir.dt.float32r
bf16 = mybir.dt.bfloat16
Exp = mybir.ActivationFunctionType.Exp
AluAdd = mybir.AluOpType.add


def build_nc():
    nc = bacc.Bacc(None, target_bir_lowering=False)
    xt_d = nc.declare_dram_parameter("xt", [DIM, T], bf16, isOutput=False)
    wqk_d = nc.declare_dram_parameter("wqk", [DIM, 2 * CQ], bf16, isOutput=False)
    wv_d = nc.declare_dram_parameter("wv", [DIM, CQ], bf16, isOutput=False)
    wo_d = nc.declare_dram_parameter("wo", [CQ, DIM], bf16, isOutput=False)
    mv_d = nc.declare_dram_parameter("maskv", [128, NT], f32, isOutput=False)
    out_d = nc.declare_dram_parameter("out", [T, DIM], f32, isOutput=True)

    with tile.TileContext(nc) as tc:
        with tc.tile_pool(name="pp", bufs=1) as pp, \
             tc.tile_pool(name="xq_p", bufs=2) as xq_p, \
             tc.tile_pool(name="qtp", bufs=2) as qtp, \
             tc.tile_pool(name="p_p", bufs=6) as p_p, \
             tc.tile_pool(name="at_p", bufs=2) as at_p, \
             tc.tile_pool(name="den_p", bufs=2) as den_p, \
             tc.tile_pool(name="bcs_p", bufs=2) as bcs_p, \
             tc.tile_pool(name="acc_p", bufs=1) as acc_p, \
             tc.tile_pool(name="out_p", bufs=2) as out_p, \
             tc.tile_pool(name="ps_aux", bufs=2, space="PSUM") as ps_aux, \
             tc.tile_pool(name="ps_s", bufs=2, space="PSUM") as ps_s, \
             tc.tile_pool(name="ps_pv", bufs=1, space="PSUM") as ps_pv:

            # ---- constants ----
            mv_sb = pp.tile([128, NT], f32, name="maskv_sb", tag="maskv_sb")
            nc.scalar.dma_start(out=mv_sb, in_=mv_d[:, :])
            # one 128x128 causal strip: keep where q_local >= k_local
            dstrip = pp.tile([128, 128], f32, name="dstrip", tag="dstrip")
            nc.gpsimd.memset(dstrip, 1.0)
            nc.gpsimd.affine_select(
                out=dstrip, in_=dstrip, compare_op=mybir.AluOpType.is_ge,
                fill=0.0, base=0, pattern=[[1, 128]], channel_multiplier=-1)
            onescol = pp.tile([128, HG], f32, name="onescol", tag="onescol")
            nc.vector.memset(onescol, 1.0)
            # head-half selector rows for the PE denominator broadcast:
            # bmh[0, 0:128] selects partitions 0-63, bmh[0, 128:256] selects
            # partitions 64-127 (K=1 matmuls: out[p, q] = bmh[p] * den[q])
            bmh = pp.tile([1, 256], bf16, name="bmh", tag="bmh")
            nc.vector.memset(bmh, 0.0)
            nc.vector.memset(bmh[0:1, 0:64], 1.0)
            nc.vector.memset(bmh[0:1, 192:256], 1.0)

            # ---- persistent weights: one big DMA each ----
            wqk_sb = pp.tile([128, 8, 2 * CQ], bf16, name="wqk_sb", tag="wqk_sb")
            wv_sb = pp.tile([128, 8, CQ], bf16, name="wv_sb", tag="wv_sb")
            wo_sb = pp.tile([128, 4, DIM], bf16, name="wo_sb", tag="wo_sb")
            # x quarter tiles [128, 8 kb chunks, 512]; first quarter loads now
            xts_cur = {}

            def load_x_quarter(q):
                xq = xq_p.tile([128, 8, 512], bf16, name=f"xq{q}", tag="xq")
                x3 = xt_d[:, q * 512:(q + 1) * 512].rearrange(
                    "(kb p) t -> p kb t", p=128)
                nc.sync.dma_start(out=xq[:, 0:4, :], in_=x3[:, 0:4, :])
                nc.scalar.dma_start(out=xq[:, 4:8, :], in_=x3[:, 4:8, :])
                xts_cur[q] = xq

            wqk3 = wqk_d.rearrange("(kb p) n -> p kb n", p=128)
            nc.sync.dma_start(out=wqk_sb[:, 0:4, :], in_=wqk3[:, 0:4, :])
            nc.scalar.dma_start(out=wqk_sb[:, 4:8, :], in_=wqk3[:, 4:8, :])
            load_x_quarter(0)
            nc.sync.dma_start(
                out=wv_sb, in_=wv_d.rearrange("(kb p) n -> p kb n", p=128))
            nc.sync.dma_start(
                out=wo_sb, in_=wo_d.rearrange("(m p) n -> p m n", p=128))

            # ---- persistent tensors ----
            kt = [pp.tile([128, T], bf16, name=f"kt{m}", tag=f"kt{m}") for m in range(4)]
            vaug = [pp.tile([128, HG * 65], f32r, name=f"vaug{t}", tag=f"vaug{t}")
                    for t in range(NT)]

            qt_cur = {}    # quarter -> [4 pair tiles [128, 512]]
            ats_cur = {}   # qb -> [4 pair tiles [128, 512]]

            # ---------- qkv quarter units ----------
            def qkv_units(q, with_xload=None):
                units = []
                if with_xload is not None:
                    units.append(lambda: load_x_quarter(with_xload))
                qt_cur[q] = [None] * 4

                def qk_unit(m):
                    xq = xts_cur[q]
                    pq = ps_aux.tile([128, 512], f32, name="mm", tag="aux")
                    for kb in range(8):
                        nc.tensor.matmul(
                            pq, wqk_sb[:, kb, m * 128:(m + 1) * 128], xq[:, kb, :],
                            start=(kb == 0), stop=(kb == 7))
                    if m < 4:
                        qtile = qtp.tile([128, 512], bf16, name=f"qt{m}", tag=f"qt{m}")
                        nc.vector.tensor_copy(qtile, pq)
                        qt_cur[q][m] = qtile
                    else:
                        nc.vector.tensor_copy(
                            kt[m - 4][:, q * 512:(q + 1) * 512], pq)
                for m in range(8):
                    units.append(lambda m=m: qk_unit(m))

                def v_unit(ti):
                    xq = xts_cur[q]
                    pv = ps_aux.tile([128, 512], f32, name="mm", tag="aux")
                    for kb in range(8):
                        nc.tensor.matmul(
                            pv, xq[:, kb, ti * 128:(ti + 1) * 128], wv_sb[:, kb, :],
                            start=(kb == 0), stop=(kb == 7))
                    vt = vaug[q * 4 + ti]
                    vt3 = vt.rearrange("p (h w) -> p h w", w=65)
                    nc.vector.tensor_copy(
                        vt3[:, :, 0:64], pv.rearrange("p (h w) -> p h w", w=64))
                    nc.vector.tensor_copy(
                        vt3[:, :, 64:65], onescol.rearrange("p (h w) -> p h w", w=1))
                    nc.vector.tensor_scalar_mul(
                        vt, vt, mv_sb[:, (q * 4 + ti):(q * 4 + ti + 1)])
                for ti in range(4):
                    units.append(lambda ti=ti: v_unit(ti))
                return units

            # ---------- out_proj units ----------
            ob_cur = {}

            def op_mm(po, qb, m, ti, nb, start, stop):
                nc.tensor.matmul(
                    po, ats_cur[qb][m][:, ti * 128:(ti + 1) * 128],
                    wo_sb[:, m, nb * 512:(nb + 1) * 512],
                    start=start, stop=stop)

            def op_store(qb, ti, ob):
                t0 = (qb * 4 + ti) * 128
                nc.sync.dma_start(out=out_d[t0:t0 + 128, :], in_=ob)

            def outproj_units(qb):
                units = []

                def op_unit(ti, nb):
                    po = ps_aux.tile([128, 512], f32, name="mm", tag="aux")
                    for m in range(4):
                        op_mm(po, qb, m, ti, nb, start=(m == 0), stop=(m == 3))
                    if nb == 0:
                        ob = out_p.tile([128, DIM], f32, name="ob", tag="ob")
                        ob_cur[qb, ti] = ob
                    ob = ob_cur[qb, ti]
                    nc.vector.tensor_copy(ob[:, nb * 512:(nb + 1) * 512], po)
                    if nb == 1:
                        op_store(qb, ti, ob)
                for ti in range(4):
                    for nb in range(2):
                        units.append(lambda ti=ti, nb=nb: op_unit(ti, nb))
                return units

            # last q-block: heads 0-5 (m=0..2) accumulated into SBUF before the
            # final pair normalizes; m=3 joins after.
            acc_sb = {}

            def op_partial_units(qb):
                units = []

                def part_unit(ti, nb):
                    po = ps_aux.tile([128, 512], f32, name="mm", tag="aux")
                    for m in range(3):
                        op_mm(po, qb, m, ti, nb, start=(m == 0), stop=(m == 2))
                    acc = acc_p.tile([128, 512], f32, name=f"acc{ti}{nb}",
                                     tag=f"acc{ti}{nb}")
                    nc.vector.tensor_copy(acc, po)
                    acc_sb[ti, nb] = acc
                for ti in range(4):
                    for nb in range(2):
                        units.append(lambda ti=ti, nb=nb: part_unit(ti, nb))
                return units

            def op_final_units(qb):
                units = []

                def fin_unit(ti, nb):
                    po = ps_aux.tile([128, 512], f32, name="mm", tag="aux")
                    op_mm(po, qb, 3, ti, nb, start=True, stop=True)
                    if nb == 0:
                        ob = out_p.tile([128, DIM], f32, name="ob", tag="ob")
                        ob_cur[qb, ti] = ob
                    ob = ob_cur[qb, ti]
                    nc.vector.tensor_tensor(
                        ob[:, nb * 512:(nb + 1) * 512], po, acc_sb[ti, nb], AluAdd)
                    if nb == 1:
                        op_store(qb, ti, ob)
                for ti in range(4):
                    for nb in range(2):
                        units.append(lambda ti=ti, nb=nb: fin_unit(ti, nb))
                return units

            # ---------- attention pair ----------
            def att_pair(qb, m, pump):
                nk = 4 * (qb + 1)
                pvp = ps_pv.tile([65, 1024], f32, name="pv", tag="pv")

                def pv_mms(pk, pt, w0, stop):
                    # masked q-columns [0:w0) of this k-tile are exactly zero:
                    # skip them; PSUM accumulation keeps their prior value
                    nc.tensor.matmul(
                        pvp[:, w0:512],
                        vaug[pk][:, (2 * m) * 65:(2 * m + 1) * 65],
                        pt[:, w0:512], start=(pk == 0), stop=stop)
                    nc.tensor.matmul(
                        pvp[:, 512 + w0:1024],
                        vaug[pk][:, (2 * m + 1) * 65:(2 * m + 2) * 65],
                        pt[:, 512 + w0:1024], start=(pk == 0), stop=stop)

                pending = []
                for kti in range(nk):
                    j = kti - 4 * qb
                    w0 = 128 * j if j > 0 else 0
                    sp = ps_s.tile([128, 1024], f32, name="s", tag="s")
                    nc.tensor.matmul(
                        sp[:, w0:512],
                        kt[m][0:64, kti * 128:(kti + 1) * 128],
                        qt_cur[qb][m][0:64, w0:512], start=True, stop=True)
                    nc.tensor.matmul(
                        sp[:, 512 + w0:1024],
                        kt[m][64:128, kti * 128:(kti + 1) * 128],
                        qt_cur[qb][m][64:128, w0:512], start=True, stop=True)
                    pump()
                    ppt = p_p.tile([128, 1024], f32r, name="p", tag="p")
                    p3 = ppt.rearrange("p (h w) -> p h w", w=512)
                    s3 = sp.rearrange("p (h w) -> p h w", w=512)
                    if j < 0:
                        nc.scalar.activation(p3, s3, Exp, scale=0.125)
                    else:
                        nc.scalar.activation(
                            p3[:, :, w0:512], s3[:, :, w0:512], Exp, scale=0.125)
                        for h in range(2):
                            nc.vector.tensor_mul(
                                ppt[:, h * 512 + w0:h * 512 + w0 + 128],
                                ppt[:, h * 512 + w0:h * 512 + w0 + 128],
                                dstrip)
                    # lag-2: delay each PV by two k-tiles so the previous
                    # pair's PSUM drain has slack before our start=True write
                    pending.append((kti, ppt, w0))
                    if len(pending) > 2:
                        pv_mms(*pending.pop(0), stop=False)
                    pump()
                while pending:
                    pv_mms(*pending.pop(0), stop=(not pending))

                atm = at_p.tile([128, 512], bf16, name=f"at{m}", tag=f"at{m}")
                ats_cur[qb][m] = atm
                # the last pair's chain is the kernel tail: route its copies
                # through the (then idle) scalar engine to cut latency
                cp = nc.scalar.copy if (qb == NQ - 1 and m == 3) else \
                    nc.vector.tensor_copy
                cp(atm[0:64, :], pvp[0:64, 0:512])
                cp(atm[64:128, :], pvp[0:64, 512:1024])
                # denominator: partition-broadcast via two K=1 PE matmuls
                # (bmh half ^T @ den half), then reciprocal at 128 lanes
                denb = den_p.tile([1, 1024], bf16, name="denb", tag="denb")
                cp(denb, pvp[64:65, :])
                bfull = ps_s.tile([128, 1024], f32, name="s", tag="s")
                for h in range(2):
                    nc.tensor.matmul(
                        bfull[:, 0:512], bmh[0:1, h * 128:(h + 1) * 128],
                        denb[0:1, h * 512:(h + 1) * 512],
                        start=(h == 0), stop=(h == 1))
                bcs = bcs_p.tile([128, 512], f32, name="bcs", tag="bcs")
                nc.vector.reciprocal_approx_fast(out=bcs, in_=bfull[:, 0:512])
                nc.vector.tensor_mul(atm, atm, bcs)

            def run_phase(tasks, fillers, n_units):
                """tasks: closures taking pump(); fillers pumped proportionally."""
                nf = len(fillers)
                state = {"fi": 0, "ai": 0}

                def pump():
                    state["ai"] += 1
                    while state["fi"] * n_units < state["ai"] * nf \
                            and state["fi"] < nf:
                        fillers[state["fi"]]()
                        state["fi"] += 1
                for t in tasks:
                    t(pump)
                while state["fi"] < nf:
                    fillers[state["fi"]]()
                    state["fi"] += 1

            # ---------------- emission schedule ----------------
            for u in qkv_units(0):
                u()
            for qb in range(NQ):
                ats_cur[qb] = [None] * 4

            def phase_tasks(qb, ms):
                def mk(m):
                    def t(pump):
                        att_pair(qb, m, pump)
                    return t
                return [mk(m) for m in ms]

            # phase 0: att(0) + qkv(1)
            run_phase(phase_tasks(0, range(4)),
                      qkv_units(1, with_xload=1), 32)
            # phase 1: att(1) + op(0) + qkv(2)
            run_phase(phase_tasks(1, range(4)),
                      outproj_units(0) + qkv_units(2, with_xload=2), 64)
            # phase 2: att(2) + op(1) + qkv(3)
            run_phase(phase_tasks(2, range(4)),
                      outproj_units(1) + qkv_units(3, with_xload=3), 96)
            # phase 3: att(3) pairs 0-2 + op(2)
            run_phase(phase_tasks(3, range(3)),
                      outproj_units(2), 72)
            # phase 4: att(3) pair 3 + partial out_proj over pairs 0-2
            run_phase(phase_tasks(3, [3]),
                      op_partial_units(3), 32)
            # tail: only the m=3 out_proj slices depend on the last pair
            for u in op_final_units(3):
                u()
    nc.finalize()
    return nc


_NC_CACHE = {}


def _get_nc():
    if "nc" not in _NC_CACHE:
        _NC_CACHE["nc"] = build_nc()
    return _NC_CACHE["nc"]


def _make_in_maps(x, w_qkv, w_out, attn_mask):
    np_bf16 = mybir.dt.np(bf16)
    x = np.asarray(x, dtype=np.float32)
    w_qkv = np.asarray(w_qkv, dtype=np.float32)
    w_out = np.asarray(w_out, dtype=np.float32)
    am = np.asarray(attn_mask)
    in_maps = []
    for c in range(NCORES):
        b, hg = c // 2, c % 2
        wqk_c = np.ascontiguousarray(np.concatenate(
            [w_qkv[:, hg * CQ:(hg + 1) * CQ],
             w_qkv[:, DIM + hg * CQ:DIM + (hg + 1) * CQ]], axis=1)).astype(np_bf16)
        wv_c = np.ascontiguousarray(
            w_qkv[:, 2 * DIM + hg * CQ:2 * DIM + (hg + 1) * CQ]).astype(np_bf16)
        wo_c = np.ascontiguousarray(w_out[hg * CQ:(hg + 1) * CQ, :]).astype(np_bf16)
        mv_c = np.ascontiguousarray(
            am[b].astype(np.float32).reshape(NT, 128).T)
        xt_c = np.ascontiguousarray(x[b].T).astype(np_bf16)
        in_maps.append({
            "xt": xt_c,
            "wqk": wqk_c,
            "wv": wv_c,
            "wo": wo_c,
            "maskv": mv_c,
        })
    return in_maps


def run(x, w_qkv, w_out, attn_mask, trace=False):
    nc = _get_nc()
    in_maps = _make_in_maps(x, w_qkv, w_out, attn_mask)
    res = run_bass_kernel_spmd(nc, in_maps, list(range(NCORES)), trace=trace)
    outs = [res.results[c]["out"] for c in range(NCORES)]
    full = np.stack([outs[2 * b] + outs[2 * b + 1] for b in range(B)], axis=0)
    return full.astype(np.float32), res


def kernel(x, w_qkv, w_out, attn_mask):
    full, _ = run(x, w_qkv, w_out, attn_mask, trace=False)
    return full
